# revision 10
# baseline (speedup 1.0000x reference)
"""Trainium2 Bass kernel for a dense transformer block (nn_Block_25366076850386).

Sharding (8 cores): core c -> batch b = c//2, head-half hh = c%2.
Each core computes LN1+QKV+attention for its 8 heads over its full batch,
AllGathers attention outputs within the (2b, 2b+1) pair, computes the full
attention projection + residual, spills r1, then FFN with the FF dim split
in half per core. Host sums the pair's partial outputs:
    out[b] = part[2b] + part[2b+1],  part = 0.5*r1 + ffn_half(r1)

All GEMMs run in fp32r (full PE rate, ~1.5e-4 rel err); everything else fp32.
"""

import numpy as np

import concourse.bass as bass
import concourse.mybir as mybir
from concourse import bacc
from concourse.tile import TileContext
from concourse.masks import make_identity
from concourse.bass_utils import run_bass_kernel_spmd

F32 = mybir.dt.float32
F32R = mybir.dt.float32r
AF = mybir.ActivationFunctionType
ALU = mybir.AluOpType

B, T, C, H, D, FF = 4, 2048, 1024, 16, 64, 4096
HPC = H // 2          # heads per core = 8
FQ = HPC * D          # per-core q/k/v width = 512
FFH = FF // 2         # per-core FF width = 2048
NT = T // 128         # 16 token tiles
NCT = C // 128        # 8 channel tiles
NCH = T // 512        # 4 token chunks (512 each)
EPS = 1e-5

_CACHED = {}


def _build_program(has_bqk: bool, has_bv: bool, has_bfc: bool):
    nc = bacc.Bacc()

    xin = nc.dram_tensor("xin", [T, C], F32, kind="ExternalInput")
    wqt = nc.dram_tensor("wqt", [C, FQ], F32, kind="ExternalInput")
    wkt = nc.dram_tensor("wkt", [C, FQ], F32, kind="ExternalInput")
    wvt = nc.dram_tensor("wvt", [C, FQ], F32, kind="ExternalInput")
    wpt = nc.dram_tensor("wpt", [C, C], F32, kind="ExternalInput")
    wfct = nc.dram_tensor("wfct", [C, FFH], F32, kind="ExternalInput")
    wfpt = nc.dram_tensor("wfpt", [FFH, C], F32, kind="ExternalInput")
    out = nc.dram_tensor("out", [T, C], F32, kind="ExternalOutput")
    bqk_d = bv_d = bfc_d = None
    if has_bqk:
        bqk_d = nc.dram_tensor("bqk", [2, FQ], F32, kind="ExternalInput")
    if has_bv:
        bv_d = nc.dram_tensor("bv", [FQ], F32, kind="ExternalInput")
    if has_bfc:
        bfc_d = nc.dram_tensor("bfc", [FFH], F32, kind="ExternalInput")

    x_t = xin[:].rearrange("(nt p) c -> nt p c", p=128)
    wqt_r = wqt[:].rearrange("(ct p) f -> ct p f", p=128)
    wkt_r = wkt[:].rearrange("(ct p) f -> ct p f", p=128)
    wvt_r = wvt[:].rearrange("(ct p) f -> ct p f", p=128)
    wpt_r = wpt[:].rearrange("(hd p) o -> hd p o", p=128)
    wfct_r = wfct[:].rearrange("(ct p) f -> ct p f", p=128)
    wfpt_r = wfpt[:].rearrange("(ft p) c -> ft p c", p=128)
    out_t = out[:].rearrange("(nt p) c -> nt p c", p=128)

    eps_ref = []

    def layernorm(pool, xt, h_out):
        """h_out[:] = (xt - mean)/sqrt(var+eps), rowwise over free dim (C)."""
        stats = pool.tile([128, 2, 6], F32, tag="ln_stats")
        nc.vector.bn_stats(stats[:, 0, :], xt[:, 0:512])
        nc.vector.bn_stats(stats[:, 1, :], xt[:, 512:1024])
        mv = pool.tile([128, 2], F32, tag="ln_mv")
        nc.vector.bn_aggr(mv, stats)
        rstd = pool.tile([128, 1], F32, tag="ln_rstd")
        nc.scalar.activation(rstd, mv[:, 1:2], AF.Sqrt, bias=eps_ref[0])
        nc.vector.reciprocal(rstd, rstd)
        nc.vector.tensor_scalar(
            out=h_out, in0=xt, scalar1=mv[:, 0:1], scalar2=rstd,
            op0=ALU.subtract, op1=ALU.mult,
        )

    with TileContext(nc) as tc:
        with (
            tc.tile_pool(name="persist", bufs=1) as persist,
            tc.tile_pool(name="lnp", bufs=2) as lnp,
            tc.tile_pool(name="dram", bufs=1, space="DRAM") as dram,
        ):
            # --- constants ---
            ident = persist.tile([128, 128], F32, tag="ident")
            make_identity(nc, ident)
            eps_sb = persist.tile([128, 1], F32, tag="eps")
            nc.vector.memset(eps_sb, EPS)
            eps_ref.append(eps_sb)
            ones8 = persist.tile([128, HPC], F32, tag="ones8")
            nc.vector.memset(ones8, 1.0)
            bqk_sb = bv_sb = bfc_sb = None
            if has_bqk:
                bqk_sb = persist.tile([128, 2, FQ // 128], F32, tag="bqk")
                nc.gpsimd.dma_start(
                    bqk_sb, bqk_d[:].rearrange("q (g p) -> p q g", p=128))
            if has_bv:
                bv_sb = persist.tile([128, FQ // 128], F32, tag="bv")
                nc.gpsimd.dma_start(
                    bv_sb, bv_d[:].rearrange("(g p) -> p g", p=128))
            if has_bfc:
                bfc_sb = persist.tile([128, FFH // 128], F32, tag="bfc")
                nc.gpsimd.dma_start(
                    bfc_sb, bfc_d[:].rearrange("(g p) -> p g", p=128))

            # ============ attention phase, chunk by chunk ============
            agos = [dram.tile([2 * FQ, 512], F32, tag=f"ago{j}",
                              name=f"ago{j}") for j in range(NCH)]
            with (
                tc.tile_pool(name="kv", bufs=1) as kvp,
                tc.tile_pool(name="att_sb", bufs=1) as asb,
                tc.tile_pool(name="att_q", bufs=2) as aqp,
                tc.tile_pool(name="att_yc", bufs=1) as ayc,
                tc.tile_pool(name="att_x", bufs=3) as axp,
                tc.tile_pool(name="att_w", bufs=1) as awp,
                tc.tile_pool(name="att_e", bufs=3) as aep,
                tc.tile_pool(name="nrm", bufs=2) as nrm,
                tc.tile_pool(name="ps_tp", bufs=2, space="PSUM") as ps_tp,
                tc.tile_pool(name="ps_mm", bufs=2, space="PSUM") as ps_mm,
                tc.tile_pool(name="ps_s", bufs=2, space="PSUM") as ps_s,
                tc.tile_pool(name="ps_y", bufs=2, space="PSUM") as ps_y,
            ):
                # persistent K^T [4][128hd, T], V(+ones col) [16][128t, 8, 65]
                kT = [kvp.tile([128, T], F32R, tag=f"kT{g}", name=f"kT{g}")
                      for g in range(4)]
                vON = [kvp.tile([128, HPC, D + 1], F32R, tag=f"v{i}",
                                name=f"v{i}") for i in range(NT)]
                for j in range(NCH):
                    # ---- LN1 + transpose -> h1T chunk [8][128c, 512t]
                    h1T = [asb.tile([128, 512], F32R, tag=f"h1T{ct}",
                                    name=f"h1T{ct}") for ct in range(NCT)]
                    for tsub in range(4):
                        it = j * 4 + tsub
                        xt = axp.tile([128, C], F32, tag="x")
                        nc.gpsimd.dma_start(xt, x_t[it])
                        h1 = lnp.tile([128, C], F32, tag="h")
                        layernorm(lnp, xt, h1)
                        for ct in range(NCT):
                            tp = ps_tp.tile([128, 128], F32, tag="tp")
                            nc.tensor.transpose(
                                tp, h1[:, ct * 128:(ct + 1) * 128], ident)
                            nc.vector.tensor_copy(
                                h1T[ct][:, tsub * 128:(tsub + 1) * 128], tp)
                    # ---- Q,K projections for this chunk: out [f, 512t]
                    qT = [aqp.tile([128, 512], F32R, tag=f"qT{g}",
                                   name=f"qT{g}") for g in range(4)]
                    for which, wr, dst in (("q", wqt_r, qT), ("k", wkt_r, kT)):
                        wts = []
                        for ct in range(NCT):
                            wt = awp.tile([128, FQ], F32R, tag=f"w{ct}",
                                          name=f"w_{which}{ct}")
                            nc.gpsimd.dma_start(wt, wr[ct].bitcast(F32R))
                            wts.append(wt)
                        for g in range(4):
                            ps = ps_mm.tile([128, 512], F32, tag="mm")
                            for ct in range(NCT):
                                nc.tensor.matmul(
                                    ps, wts[ct][:, g * 128:(g + 1) * 128],
                                    h1T[ct], start=(ct == 0), stop=(ct == 7),
                                    skip_group_check=True)
                            if which == "q":
                                dslc = dst[g][:, :]
                            else:
                                dslc = dst[g][:, j * 512:(j + 1) * 512]
                            if has_bqk:
                                bias = bqk_sb[:, 0 if which == "q" else 1,
                                              g:g + 1]
                                nc.scalar.activation(dslc, ps, AF.Copy,
                                                     bias=bias)
                            else:
                                nc.scalar.activation(dslc, ps, AF.Copy)
                    # ---- V projection: out [128t, 512f] per t-tile
                    wvs = []
                    for ct in range(NCT):
                        wv = awp.tile([128, FQ], F32R, tag=f"wv{ct}",
                                      name=f"w_v{ct}")
                        nc.gpsimd.dma_start(wv, wvt_r[ct].bitcast(F32R))
                        wvs.append(wv)
                    for tsub in range(4):
                        it = j * 4 + tsub
                        ps = ps_mm.tile([128, 512], F32, tag="mm")
                        for ct in range(NCT):
                            nc.tensor.matmul(
                                ps, h1T[ct][:, tsub * 128:(tsub + 1) * 128],
                                wvs[ct], start=(ct == 0), stop=(ct == 7),
                                skip_group_check=True)
                        nc.vector.tensor_copy(
                            vON[it][:, :, D], ones8)
                        nc.vector.tensor_copy(
                            vON[it][:, :, 0:D],
                            ps.rearrange("p (h d) -> p h d", h=HPC))
                    # ---- attention for q-chunk j, all 8 heads
                    ycon = [ayc.tile([128, 512], F32, tag=f"yc{g}",
                                     name=f"yc{g}") for g in range(4)]
                    nkt = 4 * j + 4
                    for h in range(HPC):
                        g, poff = h // 2, (h % 2) * 64
                        yps = ps_y.tile([65, 512], F32, tag="y")
                        for kt in range(nkt):
                            sps = ps_s.tile([128, 512], F32, tag="s")
                            nc.tensor.matmul(
                                sps,
                                kT[g][poff:poff + 64,
                                      kt * 128:(kt + 1) * 128],
                                qT[g][poff:poff + 64, :],
                                start=True, stop=True, skip_group_check=True)
                            et = aep.tile([128, 512], F32R, tag="E")
                            nc.scalar.activation(et, sps, AF.Exp)
                            if kt >= 4 * j:
                                # zero out strictly-above-diagonal (q < k)
                                nc.gpsimd.affine_select(
                                    out=et, in_=et, compare_op=ALU.is_ge,
                                    fill=0.0, base=-128 * (kt - 4 * j),
                                    pattern=[[1, 512]], channel_multiplier=-1)
                            nc.tensor.matmul(
                                yps, vON[kt][:, h, :], et,
                                start=(kt == 0), stop=(kt == nkt - 1),
                                skip_group_check=True)
                        recip = nrm.tile([1, 512], F32, tag="recip")
                        nc.vector.reciprocal(recip, yps[64:65, :])
                        bc = nrm.tile([64, 512], F32, tag="bc")
                        nc.gpsimd.partition_broadcast(bc, recip)
                        nc.vector.tensor_tensor(
                            out=ycon[g][poff:poff + 64, :],
                            in0=yps[0:64, :], in1=bc, op=ALU.mult)
                        if has_bv:
                            nc.vector.tensor_scalar_add(
                                out=ycon[g][poff:poff + 64, :],
                                in0=ycon[g][poff:poff + 64, :],
                                scalar1=bv_sb[poff:poff + 64, g:g + 1])
                    # ---- AllGather y within the pair -> ago[j] in DRAM
                    agi = dram.tile([FQ, 512], F32, tag=f"agi{j}",
                                    name=f"agi{j}")
                    for g in range(4):
                        nc.sync.dma_start(
                            agi[g * 128:(g + 1) * 128, :], ycon[g])
                    nc.gpsimd.collective_compute(
                        "AllGather", ALU.bypass,
                        replica_groups=[[0, 1], [2, 3], [4, 5], [6, 7]],
                        ins=[agi[:]], outs=[agos[j][:]])

            # ====== FFN phase: proj + residual + LN2 + FFN-half ======
            with (
                tc.tile_pool(name="ffn_wp", bufs=1) as fwpp,
                tc.tile_pool(name="ffn_yf", bufs=2) as fyf,
                tc.tile_pool(name="ffn_x", bufs=3) as ffx,
                tc.tile_pool(name="ffn_sb", bufs=1) as fsb,
                tc.tile_pool(name="ffn_g", bufs=1) as fgp,
                tc.tile_pool(name="ffn_r1", bufs=5) as fr1,
                tc.tile_pool(name="ffn_wfc", bufs=1) as fwc,
                tc.tile_pool(name="ffn_wfp", bufs=1) as fwp,
                tc.tile_pool(name="ffn_out", bufs=3) as fop,
                tc.tile_pool(name="ps_mm2", bufs=2, space="PSUM") as ps_mm2,
                tc.tile_pool(name="ps_tp2", bufs=2, space="PSUM") as ps_tp2,
                tc.tile_pool(name="ps_u", bufs=2, space="PSUM") as ps_u,
                tc.tile_pool(name="ps_z2", bufs=2, space="PSUM") as ps_z2,
            ):
                # attention-proj weights resident: [8][128hd, 1024o]
                wps = []
                for g8 in range(8):
                    wp_ = fwpp.tile([128, C], F32R, tag=f"wp{g8}",
                                    name=f"wp{g8}")
                    nc.gpsimd.dma_start(wp_, wpt_r[g8].bitcast(F32R))
                    wps.append(wp_)
                for j in range(NCH):
                    ago_r = agos[j][:].rearrange("(g p) q -> g p q", p=128)
                    h2T = [fsb.tile([128, 512], F32R, tag=f"h2T{ct}",
                                    name=f"h2T{ct}") for ct in range(NCT)]
                    r1ts = []
                    for tsub in range(4):
                        it = j * 4 + tsub
                        # proj: z[tsub] = yfull^T.T @ wp ; r1 = x + z
                        yfs = []
                        for g8 in range(8):
                            yf = fyf.tile([128, 128], F32R, tag=f"yf{g8}",
                                          name=f"yf{g8}")
                            nc.gpsimd.dma_start(
                                yf,
                                ago_r[g8][:, tsub * 128:(tsub + 1) * 128]
                                .bitcast(F32R))
                            yfs.append(yf)
                        x2 = ffx.tile([128, C], F32, tag="x2")
                        nc.gpsimd.dma_start(x2, x_t[it])
                        r1t = fr1.tile([128, C], F32, tag="fr1")
                        r1ts.append(r1t)
                        for nchk in range(2):
                            zps = ps_mm2.tile([128, 512], F32, tag="mm2")
                            for g8 in range(8):
                                nc.tensor.matmul(
                                    zps, yfs[g8],
                                    wps[g8][:, nchk * 512:(nchk + 1) * 512],
                                    start=(g8 == 0), stop=(g8 == 7),
                                    skip_group_check=True)
                            nc.vector.tensor_tensor(
                                out=r1t[:, nchk * 512:(nchk + 1) * 512],
                                in0=zps,
                                in1=x2[:, nchk * 512:(nchk + 1) * 512],
                                op=ALU.add)
                        h2 = lnp.tile([128, C], F32, tag="h")
                        layernorm(lnp, r1t, h2)
                        for ct in range(NCT):
                            tp = ps_tp2.tile([128, 128], F32, tag="tp2")
                            nc.tensor.transpose(
                                tp, h2[:, ct * 128:(ct + 1) * 128], ident)
                            nc.vector.tensor_copy(
                                h2T[ct][:, tsub * 128:(tsub + 1) * 128], tp)
                    # fc + gelu -> g tiles [16][128f, 512t], wfc in quarters
                    gts = []
                    for fh in range(4):
                        wfcs = []
                        for ct in range(NCT):
                            wf = fwc.tile([128, 512], F32R, tag=f"wfc{ct}",
                                          name=f"wfc{ct}")
                            nc.gpsimd.dma_start(
                                wf,
                                wfct_r[ct][:, fh * 512:(fh + 1) * 512]
                                .bitcast(F32R))
                            wfcs.append(wf)
                        for fl in range(4):
                            ft = fh * 4 + fl
                            ups = ps_u.tile([128, 512], F32, tag="u")
                            for ct in range(NCT):
                                nc.tensor.matmul(
                                    ups, wfcs[ct][:, fl * 128:(fl + 1) * 128],
                                    h2T[ct], start=(ct == 0), stop=(ct == 7),
                                    skip_group_check=True)
                            gt = fgp.tile([128, 512], F32R, tag=f"g{ft}",
                                          name=f"g{ft}")
                            if has_bfc:
                                nc.scalar.activation(
                                    gt, ups, AF.Gelu,
                                    bias=bfc_sb[:, ft:ft + 1])
                            else:
                                nc.scalar.activation(gt, ups, AF.Gelu)
                            gts.append(gt)
                    # fc_proj partial + 0.5*r1 -> out, wfp streamed in halves
                    for nchk in range(2):
                        wfph = []
                        for ft in range(16):
                            wf = fwp.tile([128, 512], F32R, tag=f"wfp{ft}",
                                          name=f"wfp{ft}")
                            nc.gpsimd.dma_start(
                                wf,
                                wfpt_r[ft][:, nchk * 512:(nchk + 1) * 512]
                                .bitcast(F32R))
                            wfph.append(wf)
                        for tsub in range(4):
                            it = j * 4 + tsub
                            zps = ps_z2.tile([128, 512], F32, tag="z2")
                            for ft in range(16):
                                nc.tensor.matmul(
                                    zps,
                                    gts[ft][:, tsub * 128:(tsub + 1) * 128],
                                    wfph[ft],
                                    start=(ft == 0), stop=(ft == 15),
                                    skip_group_check=True)
                            ot = fop.tile([128, 512], F32, tag="ot")
                            nc.vector.scalar_tensor_tensor(
                                out=ot,
                                in0=r1ts[tsub][:,
                                               nchk * 512:(nchk + 1) * 512],
                                scalar=0.5, in1=zps,
                                op0=ALU.mult, op1=ALU.add)
                            nc.sync.dma_start(
                                out_t[it][:, nchk * 512:(nchk + 1) * 512], ot)

    nc.finalize()
    return nc


def _get_program(has_bqk, has_bv, has_bfc):
    key = (has_bqk, has_bv, has_bfc)
    if key not in _CACHED:
        _CACHED[key] = _build_program(*key)
    return _CACHED[key]


def _prep(x, ln1_w, ln1_b, ln2_w, ln2_b, w_attn, w_proj, w_fc, w_fc_proj,
          **unused):
    x = np.asarray(x, np.float32)
    ln1_w = np.asarray(ln1_w, np.float32)
    ln1_b = np.asarray(ln1_b, np.float32)
    ln2_w = np.asarray(ln2_w, np.float32)
    ln2_b = np.asarray(ln2_b, np.float32)
    w_attn = np.asarray(w_attn, np.float32)
    w_proj = np.asarray(w_proj, np.float32)
    w_fc = np.asarray(w_fc, np.float32)
    w_fc_proj = np.asarray(w_fc_proj, np.float32)

    scale = 1.0 / np.sqrt(D)
    in_maps = []
    bqk_all, bv_all, bfc_all = [], [], []
    for c in range(8):
        b, hh = c // 2, c % 2
        qr = slice(hh * FQ, (hh + 1) * FQ)
        kr = slice(C + hh * FQ, C + (hh + 1) * FQ)
        vr = slice(2 * C + hh * FQ, 2 * C + (hh + 1) * FQ)
        fr = slice(hh * FFH, (hh + 1) * FFH)
        wq = w_attn[qr] * ln1_w * scale
        wk = w_attn[kr] * ln1_w
        wv = w_attn[vr] * ln1_w
        bq = (w_attn[qr] @ ln1_b) * scale
        bk = w_attn[kr] @ ln1_b
        bv = w_attn[vr] @ ln1_b
        wfc_h = w_fc[fr] * ln2_w
        bfc = w_fc[fr] @ ln2_b
        m = {
            "xin": np.ascontiguousarray(x[b]),
            "wqt": np.ascontiguousarray(wq.T),
            "wkt": np.ascontiguousarray(wk.T),
            "wvt": np.ascontiguousarray(wv.T),
            "wpt": np.ascontiguousarray(w_proj.T),
            "wfct": np.ascontiguousarray(wfc_h.T),
            "wfpt": np.ascontiguousarray(w_fc_proj[:, fr].T),
        }
        bqk_all.append(np.stack([bq, bk]))
        bv_all.append(bv)
        bfc_all.append(bfc)
        in_maps.append(m)

    has_bqk = any(np.abs(a).max() > 0 for a in bqk_all)
    has_bv = any(np.abs(a).max() > 0 for a in bv_all)
    has_bfc = any(np.abs(a).max() > 0 for a in bfc_all)
    for c in range(8):
        if has_bqk:
            in_maps[c]["bqk"] = np.ascontiguousarray(bqk_all[c])
        if has_bv:
            in_maps[c]["bv"] = np.ascontiguousarray(bv_all[c])
        if has_bfc:
            in_maps[c]["bfc"] = np.ascontiguousarray(bfc_all[c])
    return in_maps, (has_bqk, has_bv, has_bfc)


def kernel(**inputs):
    in_maps, flags = _prep(**inputs)
    nc = _get_program(*flags)
    res = run_bass_kernel_spmd(nc, in_maps, list(range(8))).results

    outp = np.empty((B, T, C), np.float32)
    for b in range(B):
        outp[b] = res[2 * b]["out"] + res[2 * b + 1]["out"]
    return outp


# revision 17
# speedup vs baseline: 8.1663x; 8.1663x over previous
"""Trainium2 Bass kernel for a dense transformer block (nn_Block_25366076850386).

Sharding (8 cores): core c -> batch b = c//2, head-half hh = c%2.
Each core computes LN1+QKV+attention for its 8 heads over its full batch,
AllGathers attention outputs within the (2b, 2b+1) pair, computes the full
attention projection + residual, spills r1, then FFN with the FF dim split
in half per core. Host sums the pair's partial outputs:
    out[b] = part[2b] + part[2b+1],  part = 0.5*r1 + ffn_half(r1)

All GEMMs run in fp32r (full PE rate, ~1.5e-4 rel err); everything else fp32.
"""

import numpy as np

import concourse.bass as bass
import concourse.mybir as mybir
from concourse import bacc
from concourse.tile import TileContext
from concourse.masks import make_identity
from concourse.bass_utils import run_bass_kernel_spmd

F32 = mybir.dt.float32
F32R = mybir.dt.float32r
AF = mybir.ActivationFunctionType
ALU = mybir.AluOpType

B, T, C, H, D, FF = 4, 2048, 1024, 16, 64, 4096
HPC = H // 2          # heads per core = 8
FQ = HPC * D          # per-core q/k/v width = 512
FFH = FF // 2         # per-core FF width = 2048
NT = T // 128         # 16 token tiles
NCT = C // 128        # 8 channel tiles
NCH = T // 512        # 4 token chunks (512 each)
EPS = 1e-5

_CACHED = {}


def _build_program(has_bqk: bool, has_bv: bool, has_bfc: bool):
    nc = bacc.Bacc()

    xin = nc.dram_tensor("xin", [T, C], F32, kind="ExternalInput")
    wqt = nc.dram_tensor("wqt", [C, FQ], F32, kind="ExternalInput")
    wkt = nc.dram_tensor("wkt", [C, FQ], F32, kind="ExternalInput")
    wvt = nc.dram_tensor("wvt", [C, FQ], F32, kind="ExternalInput")
    wpt = nc.dram_tensor("wpt", [C, C], F32, kind="ExternalInput")
    wfct = nc.dram_tensor("wfct", [C, FFH], F32, kind="ExternalInput")
    wfpt = nc.dram_tensor("wfpt", [FFH, C], F32, kind="ExternalInput")
    out = nc.dram_tensor("out", [T, C], F32, kind="ExternalOutput")
    bqk_d = bv_d = bfc_d = None
    if has_bqk:
        bqk_d = nc.dram_tensor("bqk", [2, FQ], F32, kind="ExternalInput")
    if has_bv:
        bv_d = nc.dram_tensor("bv", [FQ], F32, kind="ExternalInput")
    if has_bfc:
        bfc_d = nc.dram_tensor("bfc", [FFH], F32, kind="ExternalInput")

    x_t = xin[:].rearrange("(nt p) c -> nt p c", p=128)
    wqt_r = wqt[:].rearrange("(ct p) f -> ct p f", p=128)
    wkt_r = wkt[:].rearrange("(ct p) f -> ct p f", p=128)
    wvt_r = wvt[:].rearrange("(ct p) f -> ct p f", p=128)
    wpt_r = wpt[:].rearrange("(hd p) o -> hd p o", p=128)
    wfct_r = wfct[:].rearrange("(ct p) f -> ct p f", p=128)
    wfpt_r = wfpt[:].rearrange("(ft p) c -> ft p c", p=128)
    out_t = out[:].rearrange("(nt p) c -> nt p c", p=128)

    eps_ref = []

    def layernorm(pool, xt, h_out):
        """h_out[:] = (xt - mean)/sqrt(var+eps), rowwise over free dim (C)."""
        stats = pool.tile([128, 2, 6], F32, tag="ln_stats")
        nc.vector.bn_stats(stats[:, 0, :], xt[:, 0:512])
        nc.vector.bn_stats(stats[:, 1, :], xt[:, 512:1024])
        mv = pool.tile([128, 2], F32, tag="ln_mv")
        nc.vector.bn_aggr(mv, stats)
        rstd = pool.tile([128, 1], F32, tag="ln_rstd")
        nc.scalar.activation(rstd, mv[:, 1:2], AF.Sqrt, bias=eps_ref[0])
        nc.vector.reciprocal(rstd, rstd)
        nc.vector.tensor_scalar(
            out=h_out, in0=xt, scalar1=mv[:, 0:1], scalar2=rstd,
            op0=ALU.subtract, op1=ALU.mult,
        )

    with TileContext(nc) as tc:
        with (
            tc.tile_pool(name="persist", bufs=1) as persist,
            tc.tile_pool(name="lnp", bufs=2) as lnp,
            tc.tile_pool(name="dram", bufs=1, space="DRAM") as dram,
        ):
            # --- constants ---
            ident = persist.tile([128, 128], F32, tag="ident")
            make_identity(nc, ident)
            eps_sb = persist.tile([128, 1], F32, tag="eps")
            nc.vector.memset(eps_sb, EPS)
            eps_ref.append(eps_sb)
            ones8 = persist.tile([128, HPC], F32, tag="ones8")
            nc.vector.memset(ones8, 1.0)
            ones64 = persist.tile([1, 64], F32R, tag="ones64")
            nc.vector.tensor_copy(ones64, ones8[0:1, 0:1].broadcast_to([1, 64]))
            masks = []
            for r in range(4):
                mk = persist.tile([128, 512], F32, tag=f"mask{r}",
                                  name=f"mask{r}")
                nc.gpsimd.memset(mk, 0.0)
                # additive causal mask: 0 where q >= k else -1e30
                nc.gpsimd.affine_select(
                    out=mk, in_=mk, compare_op=ALU.is_ge, fill=-1.0e30,
                    base=-128 * r, pattern=[[1, 512]], channel_multiplier=-1)
                masks.append(mk)
            bqk_sb = bv_sb = bfc_sb = None
            if has_bqk:
                bqk_sb = persist.tile([128, 2, FQ // 128], F32, tag="bqk")
                nc.sync.dma_start(
                    bqk_sb, bqk_d[:].rearrange("q (g p) -> p q g", p=128))
            if has_bv:
                bv_sb = persist.tile([128, FQ // 128], F32, tag="bv")
                nc.sync.dma_start(
                    bv_sb, bv_d[:].rearrange("(g p) -> p g", p=128))
            if has_bfc:
                bfc_sb = persist.tile([128, FFH // 128], F32, tag="bfc")
                nc.sync.dma_start(
                    bfc_sb, bfc_d[:].rearrange("(g p) -> p g", p=128))

            # ============ attention phase, chunk by chunk ============
            agos = [dram.tile([2 * FQ, 512], F32, tag=f"ago{j}",
                              name=f"ago{j}") for j in range(NCH)]
            with (
                tc.tile_pool(name="kv", bufs=1) as kvp,
                tc.tile_pool(name="att_sb", bufs=2) as asb,
                tc.tile_pool(name="att_q", bufs=2) as aqp,
                tc.tile_pool(name="att_yc", bufs=1) as ayc,
                tc.tile_pool(name="att_x", bufs=3) as axp,
                tc.tile_pool(name="att_w", bufs=1) as awp,
                tc.tile_pool(name="att_e", bufs=3) as aep,
                tc.tile_pool(name="nrm", bufs=2) as nrm,
                tc.tile_pool(name="ps_tp", bufs=1, space="PSUM") as ps_tp,
                tc.tile_pool(name="ps_mm", bufs=2, space="PSUM") as ps_mm,
                tc.tile_pool(name="ps_s", bufs=3, space="PSUM") as ps_s,
                tc.tile_pool(name="ps_y", bufs=2, space="PSUM") as ps_y,
            ):
                # persistent K^T [4][128hd, T], V(+ones col) [16][128t, 8, 65]
                kT = [kvp.tile([128, T], F32R, tag=f"kT{g}", name=f"kT{g}")
                      for g in range(4)]
                vON = [kvp.tile([128, HPC, D + 1], F32R, tag=f"v{i}",
                                name=f"v{i}") for i in range(NT)]
                for j in range(NCH):
                    # ---- LN1 + transpose -> h1T chunk [8][128c, 512t]
                    h1T = [asb.tile([128, 512], F32R, tag=f"h1T{ct}",
                                    name=f"h1T{ct}") for ct in range(NCT)]
                    for tsub in range(4):
                        it = j * 4 + tsub
                        xt = axp.tile([128, C], F32, tag="x")
                        nc.sync.dma_start(xt, x_t[it])
                        h1 = lnp.tile([128, C], F32, tag="h")
                        layernorm(lnp, xt, h1)
                        for ct in range(NCT):
                            tp = ps_tp.tile([128, 128], F32, tag="tp")
                            nc.tensor.transpose(
                                tp, h1[:, ct * 128:(ct + 1) * 128], ident)
                            nc.vector.tensor_copy(
                                h1T[ct][:, tsub * 128:(tsub + 1) * 128], tp)
                    # ---- Q,K projections for this chunk: out [f, 512t]
                    qT = [aqp.tile([128, 512], F32R, tag=f"qT{g}",
                                   name=f"qT{g}") for g in range(4)]
                    for which, wr, dst in (("q", wqt_r, qT), ("k", wkt_r, kT)):
                        wts = []
                        for ct in range(NCT):
                            wt = awp.tile([128, FQ], F32R, tag=f"w{ct}",
                                          name=f"w_{which}{ct}")
                            nc.sync.dma_start(wt, wr[ct].bitcast(F32R))
                            wts.append(wt)
                        for g in range(4):
                            ps = ps_mm.tile([128, 512], F32, tag="mm")
                            for ct in range(NCT):
                                nc.tensor.matmul(
                                    ps, wts[ct][:, g * 128:(g + 1) * 128],
                                    h1T[ct], start=(ct == 0), stop=(ct == 7),
                                    skip_group_check=True)
                            if which == "q":
                                dslc = dst[g][:, :]
                            else:
                                dslc = dst[g][:, j * 512:(j + 1) * 512]
                            if has_bqk:
                                bias = bqk_sb[:, 0 if which == "q" else 1,
                                              g:g + 1]
                                nc.scalar.activation(dslc, ps, AF.Copy,
                                                     bias=bias)
                            else:
                                nc.scalar.activation(dslc, ps, AF.Copy)
                    # ---- V projection: out [128t, 512f] per t-tile
                    wvs = []
                    for ct in range(NCT):
                        wv = awp.tile([128, FQ], F32R, tag=f"w{ct}",
                                      name=f"w_v{ct}")
                        nc.sync.dma_start(wv, wvt_r[ct].bitcast(F32R))
                        wvs.append(wv)
                    for tsub in range(4):
                        it = j * 4 + tsub
                        ps = ps_mm.tile([128, 512], F32, tag="mm")
                        for ct in range(NCT):
                            nc.tensor.matmul(
                                ps, h1T[ct][:, tsub * 128:(tsub + 1) * 128],
                                wvs[ct], start=(ct == 0), stop=(ct == 7),
                                skip_group_check=True)
                        nc.vector.tensor_copy(
                            vON[it][:, :, D], ones8)
                        nc.vector.tensor_copy(
                            vON[it][:, :, 0:D],
                            ps.rearrange("p (h d) -> p h d", h=HPC))
                    # ---- attention for q-chunk j, all 8 heads
                    ycon = [ayc.tile([128, 512], F32, tag=f"yc{g}",
                                     name=f"yc{g}") for g in range(4)]
                    nkt = 4 * j + 4
                    for h in range(HPC):
                        g, poff = h // 2, (h % 2) * 64
                        yps = ps_y.tile([65, 512], F32, tag="y")
                        for kt in range(nkt):
                            sps = ps_s.tile([128, 512], F32, tag="s")
                            nc.tensor.matmul(
                                sps,
                                kT[g][poff:poff + 64,
                                      kt * 128:(kt + 1) * 128],
                                qT[g][poff:poff + 64, :],
                                start=True, stop=True, skip_group_check=True)
                            if kt >= 4 * j:
                                nc.vector.tensor_tensor(
                                    out=sps, in0=sps,
                                    in1=masks[kt - 4 * j], op=ALU.add)
                            et = aep.tile([128, 512], F32R, tag="E")
                            nc.scalar.activation(et, sps, AF.Exp)
                            nc.tensor.matmul(
                                yps, vON[kt][:, h, :], et,
                                start=(kt == 0), stop=(kt == nkt - 1),
                                skip_group_check=True)
                        recip = nrm.tile([1, 512], F32R, tag="recip")
                        with nc.allow_low_precision(
                                reason="fp32r denom recip for PE broadcast"):
                            nc.vector.reciprocal(recip, yps[64:65, :])
                        bcps = ps_mm.tile([64, 512], F32, tag="mm")
                        nc.tensor.matmul(bcps, ones64, recip,
                                         start=True, stop=True,
                                         skip_group_check=True)
                        bc = nrm.tile([64, 512], F32, tag="bc")
                        nc.vector.tensor_copy(bc, bcps)
                        nc.vector.tensor_tensor(
                            out=ycon[g][poff:poff + 64, :],
                            in0=yps[0:64, :], in1=bc, op=ALU.mult)
                        if has_bv:
                            nc.vector.tensor_scalar_add(
                                out=ycon[g][poff:poff + 64, :],
                                in0=ycon[g][poff:poff + 64, :],
                                scalar1=bv_sb[poff:poff + 64, g:g + 1])
                    # ---- AllGather y within the pair -> ago[j] in DRAM
                    agi = dram.tile([FQ, 512], F32, tag=f"agi{j}",
                                    name=f"agi{j}")
                    for g in range(4):
                        nc.sync.dma_start(
                            agi[g * 128:(g + 1) * 128, :], ycon[g])
                    nc.gpsimd.collective_compute(
                        "AllGather", ALU.bypass,
                        replica_groups=[[0, 1], [2, 3], [4, 5], [6, 7]],
                        ins=[agi[:]], outs=[agos[j][:]])

            # ====== FFN phase: proj + residual + LN2 + FFN-half ======
            with (
                tc.tile_pool(name="ffn_wp", bufs=1) as fwpp,
                tc.tile_pool(name="ffn_yf", bufs=1) as fyf,
                tc.tile_pool(name="ffn_x", bufs=3) as ffx,
                tc.tile_pool(name="ffn_sb", bufs=1) as fsb,
                tc.tile_pool(name="ffn_g", bufs=1) as fgp,
                tc.tile_pool(name="ffn_r1", bufs=5) as fr1,
                tc.tile_pool(name="ffn_wfc", bufs=1) as fwc,
                tc.tile_pool(name="ffn_wfp", bufs=1) as fwp,
                tc.tile_pool(name="ffn_out", bufs=3) as fop,
                tc.tile_pool(name="ps_mm2", bufs=2, space="PSUM") as ps_mm2,
                tc.tile_pool(name="ps_tp2", bufs=2, space="PSUM") as ps_tp2,
                tc.tile_pool(name="ps_u", bufs=2, space="PSUM") as ps_u,
                tc.tile_pool(name="ps_z2", bufs=2, space="PSUM") as ps_z2,
            ):
                # attention-proj weights resident: [8][128hd, 1024o]
                wps = []
                for g8 in range(8):
                    wp_ = fwpp.tile([128, C], F32R, tag=f"wp{g8}",
                                    name=f"wp{g8}")
                    nc.sync.dma_start(wp_, wpt_r[g8].bitcast(F32R))
                    wps.append(wp_)
                for j in range(NCH):
                    ago_r = agos[j][:].rearrange("(g p) q -> g p q", p=128)
                    h2T = [fsb.tile([128, 512], F32R, tag=f"h2T{ct}",
                                    name=f"h2T{ct}") for ct in range(NCT)]
                    yfs = []
                    for g8 in range(8):
                        yf = fyf.tile([128, 512], F32R, tag=f"yf{g8}",
                                      name=f"yf{g8}")
                        nc.sync.dma_start(yf, ago_r[g8].bitcast(F32R))
                        yfs.append(yf)
                    r1ts = []
                    for tsub in range(4):
                        it = j * 4 + tsub
                        # proj: z[tsub] = yfull^T.T @ wp ; r1 = x + z
                        x2 = ffx.tile([128, C], F32, tag="x2")
                        nc.sync.dma_start(x2, x_t[it])
                        r1t = fr1.tile([128, C], F32, tag="fr1")
                        r1ts.append(r1t)
                        for nchk in range(2):
                            zps = ps_mm2.tile([128, 512], F32, tag="mm2")
                            for g8 in range(8):
                                nc.tensor.matmul(
                                    zps,
                                    yfs[g8][:, tsub * 128:(tsub + 1) * 128],
                                    wps[g8][:, nchk * 512:(nchk + 1) * 512],
                                    start=(g8 == 0), stop=(g8 == 7),
                                    skip_group_check=True)
                            nc.vector.tensor_tensor(
                                out=r1t[:, nchk * 512:(nchk + 1) * 512],
                                in0=zps,
                                in1=x2[:, nchk * 512:(nchk + 1) * 512],
                                op=ALU.add)
                        h2 = lnp.tile([128, C], F32, tag="h")
                        layernorm(lnp, r1t, h2)
                        for ct in range(NCT):
                            tp = ps_tp2.tile([128, 128], F32, tag="tp2")
                            nc.tensor.transpose(
                                tp, h2[:, ct * 128:(ct + 1) * 128], ident)
                            nc.vector.tensor_copy(
                                h2T[ct][:, tsub * 128:(tsub + 1) * 128], tp)
                    # fc + gelu -> g tiles [16][128f, 512t], wfc in quarters
                    gts = []
                    for fh in range(4):
                        wfcs = []
                        for ct in range(NCT):
                            wf = fwc.tile([128, 512], F32R, tag=f"wfc{ct}",
                                          name=f"wfc{ct}")
                            nc.sync.dma_start(
                                wf,
                                wfct_r[ct][:, fh * 512:(fh + 1) * 512]
                                .bitcast(F32R))
                            wfcs.append(wf)
                        for fl in range(4):
                            ft = fh * 4 + fl
                            ups = ps_u.tile([128, 512], F32, tag="u")
                            for ct in range(NCT):
                                nc.tensor.matmul(
                                    ups, wfcs[ct][:, fl * 128:(fl + 1) * 128],
                                    h2T[ct], start=(ct == 0), stop=(ct == 7),
                                    skip_group_check=True)
                            gt = fgp.tile([128, 512], F32R, tag=f"g{ft}",
                                          name=f"g{ft}")
                            if has_bfc:
                                nc.scalar.activation(
                                    gt, ups, AF.Gelu,
                                    bias=bfc_sb[:, ft:ft + 1])
                            else:
                                nc.scalar.activation(gt, ups, AF.Gelu)
                            gts.append(gt)
                    # fc_proj partial + 0.5*r1 -> out, wfp streamed in halves
                    for nchk in range(2):
                        wfph = []
                        for ft in range(16):
                            wf = fwp.tile([128, 512], F32R, tag=f"wfp{ft}",
                                          name=f"wfp{ft}")
                            nc.sync.dma_start(
                                wf,
                                wfpt_r[ft][:, nchk * 512:(nchk + 1) * 512]
                                .bitcast(F32R))
                            wfph.append(wf)
                        for tsub in range(4):
                            it = j * 4 + tsub
                            zps = ps_z2.tile([128, 512], F32, tag="z2")
                            for ft in range(16):
                                nc.tensor.matmul(
                                    zps,
                                    gts[ft][:, tsub * 128:(tsub + 1) * 128],
                                    wfph[ft],
                                    start=(ft == 0), stop=(ft == 15),
                                    skip_group_check=True)
                            ot = fop.tile([128, 512], F32, tag="ot")
                            nc.vector.scalar_tensor_tensor(
                                out=ot,
                                in0=r1ts[tsub][:,
                                               nchk * 512:(nchk + 1) * 512],
                                scalar=0.5, in1=zps,
                                op0=ALU.mult, op1=ALU.add)
                            nc.sync.dma_start(
                                out_t[it][:, nchk * 512:(nchk + 1) * 512], ot)

    nc.finalize()
    return nc


def _get_program(has_bqk, has_bv, has_bfc):
    key = (has_bqk, has_bv, has_bfc)
    if key not in _CACHED:
        _CACHED[key] = _build_program(*key)
    return _CACHED[key]


def _prep(x, ln1_w, ln1_b, ln2_w, ln2_b, w_attn, w_proj, w_fc, w_fc_proj,
          **unused):
    x = np.asarray(x, np.float32)
    ln1_w = np.asarray(ln1_w, np.float32)
    ln1_b = np.asarray(ln1_b, np.float32)
    ln2_w = np.asarray(ln2_w, np.float32)
    ln2_b = np.asarray(ln2_b, np.float32)
    w_attn = np.asarray(w_attn, np.float32)
    w_proj = np.asarray(w_proj, np.float32)
    w_fc = np.asarray(w_fc, np.float32)
    w_fc_proj = np.asarray(w_fc_proj, np.float32)

    scale = 1.0 / np.sqrt(D)
    in_maps = []
    bqk_all, bv_all, bfc_all = [], [], []
    for c in range(8):
        b, hh = c // 2, c % 2
        qr = slice(hh * FQ, (hh + 1) * FQ)
        kr = slice(C + hh * FQ, C + (hh + 1) * FQ)
        vr = slice(2 * C + hh * FQ, 2 * C + (hh + 1) * FQ)
        fr = slice(hh * FFH, (hh + 1) * FFH)
        wq = w_attn[qr] * ln1_w * scale
        wk = w_attn[kr] * ln1_w
        wv = w_attn[vr] * ln1_w
        bq = (w_attn[qr] @ ln1_b) * scale
        bk = w_attn[kr] @ ln1_b
        bv = w_attn[vr] @ ln1_b
        wfc_h = w_fc[fr] * ln2_w
        bfc = w_fc[fr] @ ln2_b
        m = {
            "xin": np.ascontiguousarray(x[b]),
            "wqt": np.ascontiguousarray(wq.T),
            "wkt": np.ascontiguousarray(wk.T),
            "wvt": np.ascontiguousarray(wv.T),
            "wpt": np.ascontiguousarray(w_proj.T),
            "wfct": np.ascontiguousarray(wfc_h.T),
            "wfpt": np.ascontiguousarray(w_fc_proj[:, fr].T),
        }
        bqk_all.append(np.stack([bq, bk]))
        bv_all.append(bv)
        bfc_all.append(bfc)
        in_maps.append(m)

    has_bqk = any(np.abs(a).max() > 0 for a in bqk_all)
    has_bv = any(np.abs(a).max() > 0 for a in bv_all)
    has_bfc = any(np.abs(a).max() > 0 for a in bfc_all)
    for c in range(8):
        if has_bqk:
            in_maps[c]["bqk"] = np.ascontiguousarray(bqk_all[c])
        if has_bv:
            in_maps[c]["bv"] = np.ascontiguousarray(bv_all[c])
        if has_bfc:
            in_maps[c]["bfc"] = np.ascontiguousarray(bfc_all[c])
    return in_maps, (has_bqk, has_bv, has_bfc)


def kernel(**inputs):
    in_maps, flags = _prep(**inputs)
    nc = _get_program(*flags)
    res = run_bass_kernel_spmd(nc, in_maps, list(range(8))).results

    outp = np.empty((B, T, C), np.float32)
    for b in range(B):
        outp[b] = res[2 * b]["out"] + res[2 * b + 1]["out"]
    return outp
